# revision 22
# baseline (speedup 1.0000x reference)
# LoftQ fused kernel for Trainium2 (Bass/Tile), 8-core data-parallel,
# fp8 DoubleRow main GEMM.
#
# reference:
#   W_q = (W_int - zero_point) * scale                  [out=4096, in=4096]
#   W   = W_q + (lora_B @ lora_A) * RANK**-0.5
#   y   = einsum('bsd,od->bso', x, W)                   x: [4, 2048, 4096]
#
# Strategy:
#   - Data-parallel: 8192 tokens sharded 1024/core; W replicated.
#   - The dequantized weight values (W_int - zp)*s = (s/2)*(2*W_int - 2*zp)
#     are odd integers in [-15, 15] after rescaling -> EXACTLY representable
#     in fp8 e4m3. The main GEMM runs as fp8 DoubleRow matmuls (2 MACs per
#     PE cell per cycle, K=256 per matmul): psum = x8 @ w8.T with w8 exact
#     and x8 = e4m3(x) (x-quantization error only perturbs the small W_q
#     part of W, ~0.1% of output rms).
#   - W8 tiles are the stationary operand, streamed from HBM (never
#     resident); x8 is the moving operand (resident, 4MB). Output is
#     produced transposed ([O, T]) and untransposed on the host.
#   - LoRA path stays bf16: u^T = (x @ A^T)^T on the PE (warmup pass),
#     then a K=16 bf16 tail matmul per output tile opens each PSUM
#     accumulation group (start=True) before the fp8 matmuls pile on.
#     bts is pre-scaled by SCALING*2/s so one eviction scale fits all.
#   - Eviction: psum * (s/2) -> fp16 on the Vector engine, DMA out.
#
# Host-side work is limited to sharding/layout packing (transpose + dtype
# packing); all FLOPs (both matmuls) run on device.

import numpy as np
import ml_dtypes

import concourse.bass as bass
import concourse.mybir as mybir
import concourse.tile as tile
from concourse import bacc
from concourse.bass import ts
from concourse.bass_utils import run_bass_kernel_spmd

P = 128
N_CORES = 8
RANK = 16
SCALING = RANK ** (-0.5)
BF16 = mybir.dt.bfloat16
F32 = mybir.dt.float32
F16 = mybir.dt.float16
F8 = mybir.dt.float8e4
UW = 256  # u-pass token chunk
CW = 512  # main-pass token chunk (PSUM bank width in fp32)


def build_program(nc, T, D, O, R, scale):
    """Emit the per-core program.

    T: tokens per core, D: in_features, O: out_features, R: lora rank.
    scale: quant scale s; eviction multiplies psum by s/2.
    Inputs (per core):
      xtp  bf16 [T/UW, P, D/P, UW]   x-shard bf16, u-chunk-major packing
      x8p  f8   [P, D/256, 2, T]     x-shard e4m3, DoubleRow K-pair packing
      w8p  f8   [O/P, P, D/256, 2, P]  (2*W_int - 2*zp)^T tile slabs
      atp  bf16 [P, D/P, R]          lora_A^T packed
      btsp bf16 [R, O]               lora_B^T * (SCALING*2/s)
    Output: ytp f16 [O, T]  (y transposed; host untransposes + casts f32)
    """
    DT, DTT, OT = D // P, D // 256, O // P
    UC, NCH = T // UW, T // CW
    xt = nc.dram_tensor("xtp", [UC, P, DT, UW], BF16, kind="ExternalInput")
    w8 = nc.dram_tensor("w8p", [OT, P, DTT, 2, P], F8, kind="ExternalInput")
    at = nc.dram_tensor("atp", [P, DT, R], BF16, kind="ExternalInput")
    # bts replicated at partition bases 0/32/64/96 so the K=16 LoRA tails
    # of an ot-pair can run concurrently in distinct PE row groups
    bts = nc.dram_tensor("btsp4", [P, O], BF16, kind="ExternalInput")
    y = nc.dram_tensor("ytp", [O, T], F16, kind="ExternalOutput")
    y_ap = y.ap().rearrange("(ot p) t -> ot p t", p=P)

    COPY = mybir.ActivationFunctionType.Copy
    DR = mybir.MatmulPerfMode.DoubleRow

    with tile.TileContext(nc) as tc:
        with (
            tc.tile_pool(name="const", bufs=1) as cpool,
            tc.tile_pool(name="xtpool", bufs=4) as xtpool,
            tc.tile_pool(name="w8pool", bufs=8) as w8pool,
            tc.tile_pool(name="outpool", bufs=8) as outpool,
            tc.tile_pool(name="psum", bufs=8, space="PSUM") as psum,
        ):
            at_sb = cpool.tile([P, DT, R], BF16)
            nc.sync.dma_start(at_sb[:], at.ap())
            bts_sb = cpool.tile([P, O], BF16)
            nc.sync.dma_start(bts_sb[:], bts.ap())
            # First u-chunk's x DMAs go first (8-way split -> 8 queues) so
            # the PE can start working ~10us after launch; everything else
            # queues behind them.
            xsplit = min(8, DT)
            xt_sbs = []
            for c in range(UC):
                xt_sb = xtpool.tile([P, DT, UW], BF16, tag="xt", name=f"xt_{c}")
                xt_sbs.append(xt_sb)
                for h in range(xsplit):
                    nc.sync.dma_start(
                        xt_sb[:, ts(h, DT // xsplit)],
                        xt.ap()[c, :, ts(h, DT // xsplit)],
                    )
            # x8: resident fp8 moving operand, derived on-device from xt on
            # the Vector engine (saves 4MB of startup-critical HBM traffic)
            x8_sb = cpool.tile([P, DTT, 2, T], F8)

            # u^T = (x @ A^T)^T in bf16, chunk-major packing; replicated at
            # the 4 row-group partition bases for the packed tails
            ut_sb = cpool.tile([P, T], BF16)
            for c in range(UC):
                xt_sb = xt_sbs[c]
                pu = psum.tile([P, CW], F32, tag="ps", name=f"pu_{c}")[
                    :R, :UW
                ]
                for dt in range(DT):
                    nc.tensor.matmul(
                        pu[:],
                        lhsT=at_sb[:, dt],
                        rhs=xt_sb[:, dt],
                        start=(dt == 0),
                        stop=(dt == DT - 1),
                    )
                for g in range(4):
                    nc.scalar.activation(
                        ut_sb[32 * g : 32 * g + R, ts(c, UW)], pu[:], COPY
                    )
                for dtt in range(DTT):
                    for i in range(2):
                        nc.vector.tensor_copy(
                            x8_sb[:, dtt, i, ts(c, UW)],
                            xt_sb[:, 2 * dtt + i],
                        )

            # Main GEMM over o-tile PAIRS: stream both W8 slabs from HBM,
            # open the pair's 2*NCH PSUM groups with row-group-packed bf16
            # LoRA tails (concurrent in the PE array), then 16 DoubleRow
            # K=256 matmuls per (o-tile, token-chunk).
            for otp in range(OT // 2):
                wts = []
                for k in range(2):
                    ot = 2 * otp + k
                    wt = w8pool.tile(
                        [P, DTT, 2, P], F8, tag="wt", name=f"wt_{ot}"
                    )
                    wts.append(wt)
                    nsp = min(4, DTT)
                    for h in range(nsp):
                        nc.sync.dma_start(
                            wt[:, ts(h, DTT // nsp)],
                            w8.ap()[ot, :, ts(h, DTT // nsp)],
                        )
                pss = []
                for j in range(2 * NCH):
                    k, c = j // NCH, j % NCH
                    ot = 2 * otp + k
                    ps = psum.tile([P, CW], F32, tag="ps", name=f"ps_{ot}_{c}")
                    pss.append(ps)
                    g = j % 4
                    nc.tensor.matmul(
                        ps[:],
                        lhsT=bts_sb[32 * g : 32 * g + R, ts(ot, P)],
                        rhs=ut_sb[32 * g : 32 * g + R, ts(c, CW)],
                        start=True,
                        stop=False,
                        tile_position=(32 * g, 0),
                    )
                for dtt in range(DTT):
                    for j in range(2 * NCH):
                        k, c = j // NCH, j % NCH
                        nc.tensor.matmul(
                            pss[j][:],
                            lhsT=wts[k][:, dtt],
                            rhs=x8_sb[:, dtt, :, ts(c, CW)],
                            start=False,
                            stop=(dtt == DTT - 1),
                            perf_mode=DR,
                        )
                for j in range(2 * NCH):
                    k, c = j // NCH, j % NCH
                    ot = 2 * otp + k
                    ob = outpool.tile([P, CW], F16, tag="ob", name=f"ob_{ot}_{c}")
                    nc.vector.tensor_scalar_mul(ob[:], pss[j][:], scale / 2)
                    for h in range(2):
                        nc.sync.dma_start(
                            y_ap[ot, :, ts(2 * c + h, CW // 2)],
                            ob[:, ts(h, CW // 2)],
                        )
    return nc


def _pack_inputs(x, W_int, lora_A, lora_B, s, zp):
    """Host-side shard + layout packing. Returns per-core input maps."""
    BS, S, D = x.shape
    O = W_int.shape[0]
    Tfull = BS * S
    T = Tfull // N_CORES
    DT, DTT, OT = D // P, D // 256, O // P
    UC = T // UW

    xf = np.asarray(x, dtype=np.float32).reshape(Tfull, D)
    xb = xf.astype(ml_dtypes.bfloat16)
    # w8[o, d] = 2*W_int[o, d] - 2*zp  (odd ints, exact in e4m3)
    w8 = (2.0 * np.asarray(W_int, dtype=np.float32) - 2.0 * zp).astype(
        ml_dtypes.float8_e4m3
    )
    # [ot, ki, dtt, i, j] <- w8[o=ot*P+j, d=dtt*256+i*128+ki]
    w8p = np.ascontiguousarray(
        w8.reshape(OT, P, DTT, 2, P).transpose(0, 4, 2, 3, 1)
    )
    atp = np.ascontiguousarray(
        np.asarray(lora_A, dtype=np.float32)
        .T.reshape(DT, P, RANK)
        .transpose(1, 0, 2)
        .astype(ml_dtypes.bfloat16)
    )
    bts = (np.asarray(lora_B, dtype=np.float32).T * (SCALING * 2.0 / s)).astype(
        ml_dtypes.bfloat16
    )
    btsp4 = np.zeros((P, O), dtype=ml_dtypes.bfloat16)
    for g in range(4):
        btsp4[32 * g : 32 * g + RANK] = bts
    in_maps = []
    for c in range(N_CORES):
        xs = xb[c * T : (c + 1) * T]  # [T, D] bf16
        # [uc, p, dt, t] <- x[t=uc*UW+t', d=dt*P+p]
        xtp = np.ascontiguousarray(
            xs.T.reshape(DT, P, UC, UW).transpose(2, 1, 0, 3)
        )
        in_maps.append({"xtp": xtp, "w8p": w8p, "atp": atp, "btsp4": btsp4})
    return in_maps, T, D, O


def _install_ntff_shim():
    """Provide antenv.axon_hooks (absent in this image) so that
    run_bass_kernel_spmd(trace=True) can capture NTFF profiles via the
    axon .so — mirrors trn_agent_boot.trn_boot's degraded-silently path.
    Only used for our own measurement runs (_trace=True)."""
    import sys as _sys
    import types as _types

    if "antenv.axon_hooks" in _sys.modules:
        return
    try:
        from trn_agent_boot.trn_boot import _ntff_profile_via_ctypes
    except ImportError:
        _sys.path.insert(0, "/root/.axon_site")
        from trn_agent_boot.trn_boot import _ntff_profile_via_ctypes

    hook = _ntff_profile_via_ctypes("/opt/axon/libaxon_pjrt.so")
    mod = _types.ModuleType("antenv.axon_hooks")
    mod._hook = hook
    mod.get_axon_ntff_profile_hook = lambda: mod._hook
    mod.set_axon_ntff_profile_hook = lambda h: setattr(mod, "_hook", h)
    _sys.modules["antenv.axon_hooks"] = mod
    import antenv as _antenv

    _antenv.axon_hooks = mod


def kernel(x, W_int, lora_A, lora_B, scale, zero_point, _trace=False, _tmpdir=None):
    if _trace:
        _install_ntff_shim()
    x = np.asarray(x)
    BS, S, D = x.shape
    s = float(np.asarray(scale))
    zp = float(np.asarray(zero_point))
    in_maps, T, D, O = _pack_inputs(x, W_int, lora_A, lora_B, s, zp)

    nc = bacc.Bacc(
        "TRN2",
        target_bir_lowering=False,
        debug=False,
        num_devices=N_CORES,
    )
    build_program(nc, T, D, O, RANK, scale=s)
    nc.compile()

    res = run_bass_kernel_spmd(
        nc,
        in_maps,
        core_ids=list(range(N_CORES)),
        trace=_trace,
        tmpdir=_tmpdir,
        trace_cores=list(range(N_CORES)) if _trace else None,
    )
    y = np.concatenate(
        [np.ascontiguousarray(r["ytp"].T).astype(np.float32) for r in res.results],
        axis=0,
    ).reshape(BS, S, O)
    if _trace:
        kernel.last_results = res
    return y


if __name__ == "__main__":
    # smoke: build-only for full shapes
    nc = bacc.Bacc("TRN2", target_bir_lowering=False, debug=False, num_devices=8)
    build_program(nc, 1024, 4096, 4096, 16, scale=0.01)
    nc.compile()
    print("build ok; instructions:", sum(len(b.instructions) for b in nc.main_func.blocks))


# revision 25
# speedup vs baseline: 1.0181x; 1.0181x over previous
# LoftQ fused kernel for Trainium2 (Bass/Tile), 8-core data-parallel,
# fp8 DoubleRow main GEMM.
#
# reference:
#   W_q = (W_int - zero_point) * scale                  [out=4096, in=4096]
#   W   = W_q + (lora_B @ lora_A) * RANK**-0.5
#   y   = einsum('bsd,od->bso', x, W)                   x: [4, 2048, 4096]
#
# Strategy:
#   - Data-parallel: 8192 tokens sharded 1024/core; W replicated.
#   - The dequantized weight values (W_int - zp)*s = (s/2)*(2*W_int - 2*zp)
#     are odd integers in [-15, 15] after rescaling -> EXACTLY representable
#     in fp8 e4m3. The main GEMM runs as fp8 DoubleRow matmuls (2 MACs per
#     PE cell per cycle, K=256 per matmul): psum = x8 @ w8.T with w8 exact
#     and x8 = e4m3(x) (x-quantization error only perturbs the small W_q
#     part of W, ~0.1% of output rms).
#   - W8 tiles are the stationary operand, streamed from HBM (never
#     resident); x8 is the moving operand (resident, 4MB). Output is
#     produced transposed ([O, T]) and untransposed on the host.
#   - LoRA path stays bf16: u^T = (x @ A^T)^T on the PE (warmup pass),
#     then a K=16 bf16 tail matmul per output tile opens each PSUM
#     accumulation group (start=True) before the fp8 matmuls pile on.
#     bts is pre-scaled by SCALING*2/s so one eviction scale fits all.
#   - Eviction: psum * (s/2) -> fp16 on the Vector engine, DMA out.
#
# Host-side work is limited to sharding/layout packing (transpose + dtype
# packing); all FLOPs (both matmuls) run on device.

import numpy as np
import ml_dtypes

import concourse.bass as bass
import concourse.mybir as mybir
import concourse.tile as tile
from concourse import bacc
from concourse.bass import ts
from concourse.bass_utils import run_bass_kernel_spmd

P = 128
N_CORES = 8
RANK = 16
SCALING = RANK ** (-0.5)
BF16 = mybir.dt.bfloat16
F32 = mybir.dt.float32
F16 = mybir.dt.float16
F8 = mybir.dt.float8e4
UW = 256  # u-pass token chunk
CW = 512  # main-pass token chunk (PSUM bank width in fp32)


def build_program(nc, T, D, O, R, scale):
    """Emit the per-core program.

    T: tokens per core, D: in_features, O: out_features, R: lora rank.
    scale: quant scale s; eviction multiplies psum by s/2.
    Inputs (per core):
      xtp  bf16 [T/UW, P, D/P, UW]   x-shard bf16, u-chunk-major packing
      x8p  f8   [P, D/256, 2, T]     x-shard e4m3, DoubleRow K-pair packing
      w8p  f8   [O/P, P, D/256, 2, P]  (2*W_int - 2*zp)^T tile slabs
      atp  bf16 [P, D/P, R]          lora_A^T packed
      btsp bf16 [R, O]               lora_B^T * (SCALING*2/s)
    Output: ytp f16 [O, T]  (y transposed; host untransposes + casts f32)
    """
    DT, DTT, OT = D // P, D // 256, O // P
    UC, NCH = T // UW, T // CW
    xt = nc.dram_tensor("xtp", [UC, P, DT, UW], BF16, kind="ExternalInput")
    w8 = nc.dram_tensor("w8p", [OT, P, DTT, 2, P], F8, kind="ExternalInput")
    at = nc.dram_tensor("atp", [P, DT, R], BF16, kind="ExternalInput")
    # bts replicated at partition bases 0/32/64/96 so the K=16 LoRA tails
    # of an ot-pair can run concurrently in distinct PE row groups
    bts = nc.dram_tensor("btsp4", [P, O], BF16, kind="ExternalInput")
    y = nc.dram_tensor("ytp", [O, T], F16, kind="ExternalOutput")
    y_ap = y.ap().rearrange("(ot p) t -> ot p t", p=P)

    COPY = mybir.ActivationFunctionType.Copy
    DR = mybir.MatmulPerfMode.DoubleRow

    with tile.TileContext(nc) as tc:
        with (
            tc.tile_pool(name="const", bufs=1) as cpool,
            tc.tile_pool(name="xtpool", bufs=4) as xtpool,
            tc.tile_pool(name="w8pool", bufs=8) as w8pool,
            tc.tile_pool(name="outpool", bufs=8) as outpool,
            tc.tile_pool(name="psum", bufs=8, space="PSUM") as psum,
        ):
            at_sb = cpool.tile([P, DT, R], BF16)
            nc.sync.dma_start(at_sb[:], at.ap())
            bts_sb = cpool.tile([P, O], BF16)
            nc.sync.dma_start(bts_sb[:], bts.ap())
            # First u-chunks' x DMAs go first (8-way split -> 8 queues) so
            # the PE can start working ~10us after launch; everything else
            # queues behind them in arrival-priority order.
            xsplit = min(8, DT)
            xt_sbs = [
                xtpool.tile([P, DT, UW], BF16, tag="xt", name=f"xt_{c}")
                for c in range(UC)
            ]

            def dma_xt(c):
                for h in range(xsplit):
                    nc.sync.dma_start(
                        xt_sbs[c][:, ts(h, DT // xsplit)],
                        xt.ap()[c, :, ts(h, DT // xsplit)],
                    )

            dma_xt(0)
            if UC > 1:
                dma_xt(1)
            # x8: resident fp8 moving operand, derived on-device from xt on
            # the Vector engine (saves 4MB of startup-critical HBM traffic)
            x8_sb = cpool.tile([P, DTT, 2, T], F8)

            # u^T = (x @ A^T)^T in bf16, chunk-major packing; replicated at
            # the 4 row-group partition bases for the packed tails
            ut_sb = cpool.tile([P, T], BF16)

            def emit_u_chunk(c):
                xt_sb = xt_sbs[c]
                pu = psum.tile([P, CW], F32, tag="ps", name=f"pu_{c}")[:R, :UW]
                for dt in range(DT):
                    nc.tensor.matmul(
                        pu[:],
                        lhsT=at_sb[:, dt],
                        rhs=xt_sb[:, dt],
                        start=(dt == 0),
                        stop=(dt == DT - 1),
                    )
                for g in range(4):
                    nc.scalar.activation(
                        ut_sb[32 * g : 32 * g + R, ts(c, UW)], pu[:], COPY
                    )
                for dtt in range(DTT):
                    for i in range(2):
                        nc.vector.tensor_copy(
                            x8_sb[:, dtt, i, ts(c, UW)],
                            xt_sb[:, 2 * dtt + i],
                        )

            def load_w(ot):
                wt = w8pool.tile([P, DTT, 2, P], F8, tag="wt", name=f"wt_{ot}")
                nsp = min(4, DTT)
                for h in range(nsp):
                    nc.sync.dma_start(
                        wt[:, ts(h, DTT // nsp)],
                        w8.ap()[ot, :, ts(h, DTT // nsp)],
                    )
                return wt

            def emit_groups(kots, kwts, cs):
                """One batch of PSUM groups: (ot, c) for ot in kots x c in cs.
                Opens each group with a row-group-packed bf16 LoRA tail, runs
                the DoubleRow K-loop, evicts psum*(s/2) -> fp16, DMAs out."""
                jobs = [(k, c) for c in cs for k in range(len(kots))]
                pss = []
                for g, (k, c) in enumerate(jobs):
                    ot = kots[k]
                    ps = psum.tile([P, CW], F32, tag="ps", name=f"ps_{ot}_{c}")
                    pss.append(ps)
                    nc.tensor.matmul(
                        ps[:],
                        lhsT=bts_sb[32 * g : 32 * g + R, ts(ot, P)],
                        rhs=ut_sb[32 * g : 32 * g + R, ts(c, CW)],
                        start=True,
                        stop=False,
                        tile_position=(32 * g, 0),
                    )
                for dtt in range(DTT):
                    for j, (k, c) in enumerate(jobs):
                        nc.tensor.matmul(
                            pss[j][:],
                            lhsT=kwts[k][:, dtt],
                            rhs=x8_sb[:, dtt, :, ts(c, CW)],
                            start=False,
                            stop=(dtt == DTT - 1),
                            perf_mode=DR,
                        )
                for j, (k, c) in enumerate(jobs):
                    ot = kots[k]
                    ob = outpool.tile([P, CW], F16, tag="ob", name=f"ob_{ot}_{c}")
                    nc.vector.tensor_scalar_mul(ob[:], pss[j][:], scale / 2)
                    for h in range(2):
                        nc.sync.dma_start(
                            y_ap[ot, :, ts(2 * c + h, CW // 2)],
                            ob[:, ts(h, CW // 2)],
                        )

            # Prologue: interleave the u-pass chunks with the first o-tile
            # pair, split into per-token-chunk half-batches, so the PE gets
            # main-GEMM work as soon as the first half of x has landed
            # instead of idling until the full x transfer completes.
            if NCH == 2 and OT >= 2 and UC == 4:
                emit_u_chunk(0)
                emit_u_chunk(1)
                wts0 = [load_w(0), load_w(1)]
                dma_xt(2)
                dma_xt(3)
                emit_groups([0, 1], wts0, [0])
                emit_u_chunk(2)
                emit_u_chunk(3)
                emit_groups([0, 1], wts0, [1])
                ot_start = 2
            else:
                for c in range(2, UC):
                    dma_xt(c)
                for c in range(UC):
                    emit_u_chunk(c)
                ot_start = 0

            # Main GEMM over the remaining o-tile PAIRS (c-major job order
            # so the per-matmul LDWEIGHTS alternate stationaries and stay
            # evenly spaced under the 2*NCH back-to-back streams).
            for otp in range(ot_start // 2, OT // 2):
                kots = [2 * otp, 2 * otp + 1]
                emit_groups(kots, [load_w(o) for o in kots], list(range(NCH)))
    return nc


def _pack_inputs(x, W_int, lora_A, lora_B, s, zp):
    """Host-side shard + layout packing. Returns per-core input maps."""
    BS, S, D = x.shape
    O = W_int.shape[0]
    Tfull = BS * S
    T = Tfull // N_CORES
    DT, DTT, OT = D // P, D // 256, O // P
    UC = T // UW

    xf = np.asarray(x, dtype=np.float32).reshape(Tfull, D)
    xb = xf.astype(ml_dtypes.bfloat16)
    # w8[o, d] = 2*W_int[o, d] - 2*zp  (odd ints, exact in e4m3)
    w8 = (2.0 * np.asarray(W_int, dtype=np.float32) - 2.0 * zp).astype(
        ml_dtypes.float8_e4m3
    )
    # [ot, ki, dtt, i, j] <- w8[o=ot*P+j, d=dtt*256+i*128+ki]
    w8p = np.ascontiguousarray(
        w8.reshape(OT, P, DTT, 2, P).transpose(0, 4, 2, 3, 1)
    )
    atp = np.ascontiguousarray(
        np.asarray(lora_A, dtype=np.float32)
        .T.reshape(DT, P, RANK)
        .transpose(1, 0, 2)
        .astype(ml_dtypes.bfloat16)
    )
    bts = (np.asarray(lora_B, dtype=np.float32).T * (SCALING * 2.0 / s)).astype(
        ml_dtypes.bfloat16
    )
    btsp4 = np.zeros((P, O), dtype=ml_dtypes.bfloat16)
    for g in range(4):
        btsp4[32 * g : 32 * g + RANK] = bts
    in_maps = []
    for c in range(N_CORES):
        xs = xb[c * T : (c + 1) * T]  # [T, D] bf16
        # [uc, p, dt, t] <- x[t=uc*UW+t', d=dt*P+p]
        xtp = np.ascontiguousarray(
            xs.T.reshape(DT, P, UC, UW).transpose(2, 1, 0, 3)
        )
        in_maps.append({"xtp": xtp, "w8p": w8p, "atp": atp, "btsp4": btsp4})
    return in_maps, T, D, O


def _install_ntff_shim():
    """Provide antenv.axon_hooks (absent in this image) so that
    run_bass_kernel_spmd(trace=True) can capture NTFF profiles via the
    axon .so — mirrors trn_agent_boot.trn_boot's degraded-silently path.
    Only used for our own measurement runs (_trace=True)."""
    import sys as _sys
    import types as _types

    if "antenv.axon_hooks" in _sys.modules:
        return
    try:
        from trn_agent_boot.trn_boot import _ntff_profile_via_ctypes
    except ImportError:
        _sys.path.insert(0, "/root/.axon_site")
        from trn_agent_boot.trn_boot import _ntff_profile_via_ctypes

    hook = _ntff_profile_via_ctypes("/opt/axon/libaxon_pjrt.so")
    mod = _types.ModuleType("antenv.axon_hooks")
    mod._hook = hook
    mod.get_axon_ntff_profile_hook = lambda: mod._hook
    mod.set_axon_ntff_profile_hook = lambda h: setattr(mod, "_hook", h)
    _sys.modules["antenv.axon_hooks"] = mod
    import antenv as _antenv

    _antenv.axon_hooks = mod


def kernel(x, W_int, lora_A, lora_B, scale, zero_point, _trace=False, _tmpdir=None):
    if _trace:
        _install_ntff_shim()
    x = np.asarray(x)
    BS, S, D = x.shape
    s = float(np.asarray(scale))
    zp = float(np.asarray(zero_point))
    in_maps, T, D, O = _pack_inputs(x, W_int, lora_A, lora_B, s, zp)

    nc = bacc.Bacc(
        "TRN2",
        target_bir_lowering=False,
        debug=False,
        num_devices=N_CORES,
    )
    build_program(nc, T, D, O, RANK, scale=s)
    nc.compile()

    res = run_bass_kernel_spmd(
        nc,
        in_maps,
        core_ids=list(range(N_CORES)),
        trace=_trace,
        tmpdir=_tmpdir,
        trace_cores=list(range(N_CORES)) if _trace else None,
    )
    y = np.concatenate(
        [np.ascontiguousarray(r["ytp"].T).astype(np.float32) for r in res.results],
        axis=0,
    ).reshape(BS, S, O)
    if _trace:
        kernel.last_results = res
    return y


if __name__ == "__main__":
    # smoke: build-only for full shapes
    nc = bacc.Bacc("TRN2", target_bir_lowering=False, debug=False, num_devices=8)
    build_program(nc, 1024, 4096, 4096, 16, scale=0.01)
    nc.compile()
    print("build ok; instructions:", sum(len(b.instructions) for b in nc.main_func.blocks))
